# revision 37
# baseline (speedup 1.0000x reference)
"""Trainium2 kernel for BandDecimate: scipy.signal.decimate(x, q=4, n=8,
ftype='iir', zero_phase=True) on x of shape (32, 16, 65536).

Method: filtfilt with the order-8 Chebyshev-I filter is re-expressed as a
single symmetric FIR g = corr(h, h) (h = 256-tap truncated impulse
response; slowest pole |p|=0.958 -> tail ~2e-5, far below the 2e-2
tolerance), applied to the odd-extended, steady-state-padded signal,
fused with the decimation by 4 via a 4-phase polyphase decomposition.
Each phase is a 128-tap correlation computed on the PE array as
block-Toeplitz matmuls in bf16 (halves both HBM traffic and weight-load
time vs fp32; rel-err ~3.5e-3 incl. quantization).  The right edge (last
128 decimated outputs), where the backward-pass initial condition
differs from the symmetric-FIR approximation, is computed exactly by a
dense precomputed 128x1024 linear map (8 extra matmuls).  512
independent series are sharded 64-per-core across 8 cores.

Schedule: the input V matrix streams in 8 pieces of ~1.1 MB so the first
matmul starts ~5us in (vs waiting for the whole 8.7MB chunk); compute
(8 half-passes x 16 matmuls of 512 free columns) then runs neck-and-neck
with the DMA stream.  A short burst of dummy matmuls after the edge
computation keeps the PE busy through its p-state ramp so the main
matmuls run at full clock.
"""
import os
import sys

import numpy as np

sys.path.insert(0, "/opt/trn_rl_repo")

# ---------------------------------------------------------------- constants
Q = 4
N_ORDER = 8
RP = 0.05
T = 65536
EDGE = 27
L0 = T + 2 * EDGE          # 65590
P = 256                    # truncated IIR impulse response length
NPH = 4                    # polyphase phases
PTAP = 2 * P // NPH        # 128 taps per phase
LPAD = P - 1 - EDGE        # 228 left steady-state pad of u
NB = 129                   # V column blocks (ULEN = 512*NB)
ULEN = 512 * NB            # 66048
RPAD = ULEN - LPAD - L0    # 230 right pad (clamp; fixed by edge path)
NOUT = T // Q              # 16384 outputs per series
S = 64                     # series per core
NCORES = 8
W_EDGE = 768               # edge window length
KK = W_EDGE // 128         # 6
NDELTA = 2
NIDX = NPH * NDELTA        # 8 toeplitz matrices
BLK = 8                    # output columns per psum group (free = BLK*S = 512)
HP = 8                     # half-passes (two psum groups each)
COLS_PER_HP = 2 * BLK      # 16 output n-columns per half-pass
PIECE_N = COLS_PER_HP + NDELTA - 1  # 17 V columns per piece
# half-pass 0's piece is split 9+8 columns so its first psum group starts
# one half-piece earlier; half-pass 7's likewise so its first output leaves
# while the second half still computes
PIECES = ([(0, 9), (8, 9)] + [(16 * j, 17) for j in range(1, HP - 1)]
          + [(112, 9), (120, 9)])
NWARM_TINY = 16            # p-state prewarm matmuls on framework const tiles
NWARM_CS = 6               # 256-row prewarm matmuls once consts landed

DT_MM = os.environ.get("BASS_MM_DTYPE", "bfloat16")


# ------------------------------------------------------------- filter design
def _design():
    eps = np.sqrt(10.0 ** (0.1 * RP) - 1.0)
    mu = np.arcsinh(1.0 / eps) / N_ORDER
    k = np.arange(1, N_ORDER + 1)
    theta = np.pi * (2 * k - 1) / (2 * N_ORDER)
    p = -np.sinh(mu) * np.sin(theta) + 1j * np.cosh(mu) * np.cos(theta)
    g = np.prod(-p).real
    if N_ORDER % 2 == 0:
        g /= np.sqrt(1.0 + eps**2)
    fs = 2.0
    warped = 2.0 * fs * np.tan(np.pi * (0.8 / Q) / fs)
    p = p * warped
    g = g * warped**N_ORDER
    fs2 = 2.0 * fs
    pd = (fs2 + p) / (fs2 - p)
    zd = -np.ones(N_ORDER)
    gd = g * np.real(1.0 / np.prod(fs2 - p))
    b = np.real(gd * np.poly(zd))
    a = np.real(np.poly(pd))
    n = len(a)
    comp = np.zeros((n - 1, n - 1))
    comp[0, :] = -a[1:] / a[0]
    comp[1:, :-1] = np.eye(n - 2)
    IminusA = np.eye(n - 1) - comp.T
    B = b[1:] - a[1:] * b[0]
    zi = np.linalg.solve(IminusA, B)
    return b, a, zi


def _lfilter(b, a, x, zi):
    """Direct-form II transposed; x: (T, M) float64."""
    z = zi.copy()
    y = np.empty_like(x)
    for t in range(x.shape[0]):
        xt = x[t]
        yt = b[0] * xt + z[0]
        y[t] = yt
        z = np.concatenate([z[1:], np.zeros_like(z[:1])], axis=0) \
            + b[1:, None] * xt[None, :] - a[1:, None] * yt[None, :]
    return y


def _build_weights():
    """Returns (W_main [128, NIDX, 128], W_edge [128, KK, 128]) float64."""
    b, a, zi = _design()
    x = np.zeros((P, 1))
    x[0, 0] = 1.0
    h = _lfilter(b, a, x, np.zeros((N_ORDER, 1)))[:, 0]
    g = np.correlate(h, h, mode="full")            # 2P-1 taps
    G = np.zeros(NPH * PTAP)
    G[: 2 * P - 1] = g

    # main Toeplitz matrices: lhsT[q, r*NDELTA+d, m] = G[4*(q + 128*d - m) + r]
    m = np.arange(128)
    q = np.arange(128)
    W_main = np.zeros((128, NIDX, 128))
    for r in range(NPH):
        Gr = G[r::NPH]                             # PTAP taps
        for d in range(NDELTA):
            mp = q[:, None] + 128 * d - m[None, :]  # [q, m]
            valid = (mp >= 0) & (mp < PTAP)
            W_main[:, r * NDELTA + d, :] = np.where(valid, Gr[np.clip(mp, 0, PTAP - 1)], 0.0)

    # edge matrix: exact last-128 outputs as linear map of last W_EDGE ext samples
    t_idx = np.arange(W_EDGE)
    w_idx = np.arange(W_EDGE)
    d_idx = t_idx[:, None] - w_idx[None, :]
    hmat = np.where((d_idx >= 0) & (d_idx < P), h[np.clip(d_idx, 0, P - 1)], 0.0)
    y1 = hmat                                      # [t, w] forward FIR basis
    y1_rev = y1[::-1]
    z0 = zi[:, None] * y1_rev[0][None, :]
    y2 = _lfilter(b, a, y1_rev, z0)[::-1]
    S0 = L0 - W_EDGE
    js = np.arange(NOUT - 128, NOUT)
    ts = EDGE + 4 * js - S0
    M_edge = y2[ts, :]                             # [128, W_EDGE]
    W_edge = M_edge.reshape(128, KK, 128).transpose(2, 1, 0)  # [q, kk, j]
    return W_main, W_edge


_CACHE = {}


def _np_dt():
    if DT_MM == "bfloat16":
        import ml_dtypes
        return ml_dtypes.bfloat16
    return np.float32


def _prep_static():
    if "w" not in _CACHE:
        W_main, W_edge = _build_weights()
        dt = _np_dt()
        _CACHE["w"] = (np.ascontiguousarray(W_main, dt),
                       np.ascontiguousarray(W_edge, dt))
    return _CACHE["w"]


# ------------------------------------------------------------- bass program
def _make_tile_context_cls():
    from concourse.tile import TileContext
    from concourse.vector_clock import ScopedClock, VectorClock

    class SplitDrainTileContext(TileContext):
        """This walrus build allows very few attached sync-waits per
        instruction; the stock kernel-tail drain carries one wait per DMA
        lane/engine and gets rejected.  Split it into one drain per proc,
        each with a single wait."""

        def _drain_and_barrier(self, tick_clock, wait_clock):
            gc = tick_clock.global_clock
            n = len(gc)
            for proc in range(n):
                if gc[proc] == 0:
                    continue
                vec = [0] * n
                vec[proc] = gc[proc]
                d = self.nc.sync.drain()
                wait_clock.add_sem_waits(d.ins, ScopedClock({None: VectorClock(vec)}))
            self.nc.sync.drain()
            self.nc.all_engine_barrier()
            assert self.sems is not None
            popped = self.nc._tile_sem_poison_stack.pop()
            assert popped is self._sem_poison
            self.nc.clear_and_free_semaphores(list(self.sems.allocated().values()))
            self.nc.all_engine_barrier()

    return SplitDrainTileContext


def _build_nc():
    import concourse.bass as bass
    import concourse.mybir as mybir
    TileContext = _make_tile_context_cls()

    dt_mm = getattr(mybir.dt, DT_MM)
    f32 = mybir.dt.float32

    # consts layout along free dim:
    # [wmain 8*128 | wedge 6*128 | etail 6*64 | zeros 512]
    CW = NIDX * 128 + KK * 128 + KK * S + BLK * S  # 2944
    OFF_WE = NIDX * 128
    OFF_ET = OFF_WE + KK * 128
    OFF_Z = OFF_ET + KK * S

    nc = bass.Bass(target_bir_lowering=False)
    v_d = nc.declare_dram_parameter("v", [128, NPH, NB, S], dt_mm, isOutput=False)
    c_d = nc.declare_dram_parameter("consts", [128, CW], dt_mm, isOutput=False)
    out_d = nc.declare_dram_parameter("out", [128, HP * COLS_PER_HP, S], dt_mm,
                                      isOutput=True)

    # Walrus (this version) allows at most ONE attached sync-wait per
    # instruction, so the structure below arranges that every instruction
    # has at most one un-observed dependency:
    #  - piece/osb pools are deep enough to never recycle a slot
    #  - psum groups alternate two 3-deep banks; before a recycled bank's
    #    first real matmul (which waits on the piece DMA), a zero-valued
    #    start matmul merged into the same accumulation group absorbs the
    #    slot-release (DVE) wait
    with TileContext(nc) as tc:
        with tc.tile_pool(name="const", bufs=1) as cpool, \
             tc.tile_pool(name="vchunk", bufs=len(PIECES)) as vpool, \
             tc.tile_pool(name="osb", bufs=HP) as opool, \
             tc.tile_pool(name="psum", bufs=1, space="PSUM") as ppool:

            cs = cpool.tile([128, CW], dt_mm, tag="cs")
            edge_sb = cpool.tile([128, S], dt_mm, tag="edge")

            def piece_dma(k):
                c0_, ncols_ = PIECES[k]
                pc = vpool.tile([128, NPH, ncols_, S], dt_mm, tag="chunk")
                nc.sync.dma_start(out=pc[:], in_=v_d[:, :, c0_:c0_ + ncols_, :])
                return pc

            # input stream order: the main-path weights, then half-pass 0's
            # two half-pieces, then the edge/zeros consts, then the rest —
            # so the first matmul starts as early as possible
            nc.sync.dma_start(out=cs[:, :OFF_WE], in_=c_d[:, :OFF_WE])
            pieces = [piece_dma(0), piece_dma(1)]
            nc.sync.dma_start(out=cs[:, OFF_WE:], in_=c_d[:, OFF_WE:])
            for k in range(2, len(PIECES)):
                pieces.append(piece_dma(k))

            def wm(idx):
                return cs[:, idx * 128:(idx + 1) * 128]

            def we(kk):
                return cs[:, OFF_WE + kk * 128: OFF_WE + (kk + 1) * 128]

            def et(kk):
                return cs[:, OFF_ET + kk * S: OFF_ET + (kk + 1) * S]

            # p-state prewarm: tiny matmuls on the framework const tiles run
            # during the DMA of the weights; 256-row matmuls keep the PE busy
            # until piece 0 lands
            import concourse.mybir as mybir_
            czero = nc.const_aps.aps[(mybir_.dt.float32, 0.0)]
            warm = ppool.tile([128, 256], f32, tag="warm", bufs=1)
            for _ in range(NWARM_TINY):
                nc.tensor.matmul(warm[0:1, 0:1], czero, czero,
                                 start=True, stop=True)
            for _ in range(NWARM_CS):
                nc.tensor.matmul(warm[:], wm(0), cs[:, 0:256],
                                 start=True, stop=True)
            eps = ppool.tile([128, S], f32, tag="eps", bufs=1)

            # hp j -> [(piece, local col offset) for groups A, B]
            plan = [[(pieces[0], 0), (pieces[1], 0)]]
            for j in range(1, HP - 1):
                plan.append([(pieces[j + 1], 0), (pieces[j + 1], BLK)])
            plan.append([(pieces[HP], 0), (pieces[HP + 1], 0)])

            ob = None
            for j in range(HP):
                (pcA, lbA), (pcB, lbB) = plan[j]
                psA = ppool.tile([128, BLK, S], f32, tag="psA", name=f"psA{j}",
                                 bufs=3)
                psB = ppool.tile([128, BLK, S], f32, tag="psB", name=f"psB{j}",
                                 bufs=3)
                # outputs stage in 32-col pair tiles (hp0..5) or 16-col tiles
                # (hp6/7) so the gpsimd software queue carries only 6 DMAs
                if j < 6:
                    if j % 2 == 0:
                        ob = opool.tile([128, 2 * COLS_PER_HP, S], dt_mm,
                                        tag="osb32", bufs=3)
                    off = (j % 2) * COLS_PER_HP
                else:
                    ob = opool.tile([128, COLS_PER_HP, S], dt_mm, tag="osb16",
                                    bufs=2)
                    off = 0
                recycled = j >= 3
                if recycled:
                    zs = cs[:, OFF_Z:OFF_Z + BLK * S]
                    nc.tensor.matmul(psA[:], wm(0), zs, start=True, stop=False,
                                     skip_group_check=True)
                    nc.tensor.matmul(psB[:], wm(0), zs, start=True, stop=False,
                                     skip_group_check=True)
                # hp0/hp7's halves read different pieces: emit sequentially so
                # the A chain never stalls on the B piece
                halves = ((0, psA, pcA, lbA), (1, psB, pcB, lbB))
                if j in (0, HP - 1):
                    order = [(idx, h) for h in halves for idx in range(NIDX)]
                else:
                    order = [(idx, h) for idx in range(NIDX) for h in halves]
                for idx, (half, ps, pc, lb) in order:
                    r, dd = divmod(idx, NDELTA)
                    c0 = lb + dd
                    nc.tensor.matmul(
                        ps[:], wm(idx), pc[:, r, c0:c0 + BLK, :],
                        start=(idx == 0 and not recycled),
                        stop=(idx == NIDX - 1),
                        skip_group_check=True)
                if j == 1:
                    # exact right-edge outputs (out column n=127): its consts
                    # arrive mid-stream, after half-pass 0's pieces
                    for kk in range(KK):
                        nc.tensor.matmul(eps[:], we(kk), et(kk),
                                         start=(kk == 0), stop=(kk == KK - 1))
                    nc.vector.tensor_copy(edge_sb[:], eps[:])
                nc.vector.tensor_copy(ob[:, off:off + BLK, :], psA[:])
                if j == HP - 1:
                    # first half of the final output leaves as soon as its
                    # copy lands; the rest follows after the edge column
                    nc.gpsimd.dma_start(
                        out=out_d[:, 112:112 + BLK, :], in_=ob[:, 0:BLK, :])
                nc.vector.tensor_copy(ob[:, off + BLK:off + COLS_PER_HP, :],
                                      psB[:])
                if j == HP - 1:
                    nc.vector.tensor_copy(ob[:, COLS_PER_HP - 1, :], edge_sb[:])
                    nc.gpsimd.dma_start(
                        out=out_d[:, 120:128, :], in_=ob[:, BLK:COLS_PER_HP, :])
                elif j == 6:
                    nc.gpsimd.dma_start(out=out_d[:, 96:112, :], in_=ob[:])
                elif j % 2 == 1:
                    n0 = 2 * COLS_PER_HP * (j // 2)
                    nc.gpsimd.dma_start(
                        out=out_d[:, n0:n0 + 2 * COLS_PER_HP, :], in_=ob[:])
    return nc


# --------------------------------------------------------------- host paths
def _host_prep(x):
    """x: (32, 16, T) float32 -> per-core input maps."""
    W_main, W_edge = _prep_static()
    dt = _np_dt()
    xs = np.asarray(x, np.float32).reshape(NCORES * S, T)
    left = 2.0 * xs[:, :1] - xs[:, EDGE:0:-1]
    right = 2.0 * xs[:, -1:] - xs[:, -2:-(EDGE + 2):-1]
    ext = np.concatenate([left, xs, right], axis=1)          # (512, L0)
    u = np.empty((NCORES * S, ULEN), np.float32)
    u[:, :LPAD] = ext[:, :1]
    u[:, LPAD:LPAD + L0] = ext
    u[:, LPAD + L0:] = ext[:, -1:]
    in_maps = []
    for c in range(NCORES):
        uc = u[c * S:(c + 1) * S]                            # (64, ULEN)
        # V[q, r, n, s] = u[s, 4*(q + 128*n) + r]
        u6 = uc.reshape(S, NB, 128, NPH)                     # [s, n, q, r]
        V = np.ascontiguousarray(u6.transpose(2, 3, 1, 0), dt)  # [q, r, n, s]
        etc = ext[c * S:(c + 1) * S, -W_EDGE:]               # (64, W_EDGE)
        etail = np.ascontiguousarray(
            etc.T.reshape(KK, 128, S).transpose(1, 0, 2))    # [q, kk, s]
        consts = np.concatenate(
            [W_main.reshape(128, NIDX * 128).astype(np.float32),
             W_edge.reshape(128, KK * 128).astype(np.float32),
             etail.reshape(128, KK * S),
             np.zeros((128, BLK * S), np.float32)], axis=1)  # [128, 2944]
        in_maps.append({"v": V, "consts": np.ascontiguousarray(consts, dt)})
    return in_maps


def _host_post(results):
    ys = []
    for c in range(NCORES):
        o = np.asarray(results[c]["out"], dtype=np.float32)  # [128 m, 128 n, 64 s]
        ys.append(np.ascontiguousarray(o.transpose(2, 1, 0)).reshape(S, NOUT))
    return np.concatenate(ys, axis=0).reshape(32, 16, NOUT).astype(np.float32)


def _get_nc():
    if "nc" not in _CACHE:
        _CACHE["nc"] = _build_nc()
    return _CACHE["nc"]


def kernel(x, _trace=False, _trace_kwargs=None):
    from concourse.bass_utils import run_bass_kernel_spmd
    nc = _get_nc()
    in_maps = _host_prep(x)
    res = run_bass_kernel_spmd(nc, in_maps, list(range(NCORES)),
                               trace=_trace, **(_trace_kwargs or {}))
    out = _host_post(res.results)
    if _trace:
        _CACHE["last_exec_time_ns"] = res.exec_time_ns
        _CACHE["last_result"] = res
    return out


# revision 42
# speedup vs baseline: 1.0026x; 1.0026x over previous
"""Trainium2 kernel for BandDecimate: scipy.signal.decimate(x, q=4, n=8,
ftype='iir', zero_phase=True) on x of shape (32, 16, 65536).

Method: filtfilt with the order-8 Chebyshev-I filter is re-expressed as a
single symmetric FIR g = corr(h, h) (h = 256-tap truncated impulse
response; slowest pole |p|=0.958 -> tail ~2e-5, far below the 2e-2
tolerance), applied to the odd-extended, steady-state-padded signal,
fused with the decimation by 4 via a 4-phase polyphase decomposition.
Each phase is a 128-tap correlation computed on the PE array as
block-Toeplitz matmuls in bf16 (halves both HBM traffic and weight-load
time vs fp32; rel-err ~3.5e-3 incl. quantization).  The right edge (last
128 decimated outputs), where the backward-pass initial condition
differs from the symmetric-FIR approximation, is computed exactly by a
dense precomputed 128x1024 linear map (8 extra matmuls).  512
independent series are sharded 64-per-core across 8 cores.

Schedule: the input V matrix streams in 8 pieces of ~1.1 MB so the first
matmul starts ~5us in (vs waiting for the whole 8.7MB chunk); compute
(8 half-passes x 16 matmuls of 512 free columns) then runs neck-and-neck
with the DMA stream.  A short burst of dummy matmuls after the edge
computation keeps the PE busy through its p-state ramp so the main
matmuls run at full clock.
"""
import os
import sys

import numpy as np

sys.path.insert(0, "/opt/trn_rl_repo")

# ---------------------------------------------------------------- constants
Q = 4
N_ORDER = 8
RP = 0.05
T = 65536
EDGE = 27
L0 = T + 2 * EDGE          # 65590
P = 256                    # truncated IIR impulse response length
NPH = 4                    # polyphase phases
PTAP = 2 * P // NPH        # 128 taps per phase
LPAD = P - 1 - EDGE        # 228 left steady-state pad of u
NB = 129                   # V column blocks (ULEN = 512*NB)
ULEN = 512 * NB            # 66048
RPAD = ULEN - LPAD - L0    # 230 right pad (clamp; fixed by edge path)
NOUT = T // Q              # 16384 outputs per series
S = 64                     # series per core
NCORES = 8
W_EDGE = 768               # edge window length
KK = W_EDGE // 128         # 6
NDELTA = 2
NIDX = NPH * NDELTA        # 8 toeplitz matrices
BLK = 8                    # output columns per psum group (free = BLK*S = 512)
HP = 8                     # half-passes (two psum groups each)
COLS_PER_HP = 2 * BLK      # 16 output n-columns per half-pass
PIECE_N = COLS_PER_HP + NDELTA - 1  # 17 V columns per piece
# half-pass 0's piece is split 9+8 columns so its first psum group starts
# one half-piece earlier; half-pass 7's likewise so its first output leaves
# while the second half still computes
PIECES = ([(0, 9), (8, 9)] + [(16 * j, 17) for j in range(1, HP - 1)]
          + [(112, 9), (120, 9)])
NWARM_TINY = 24            # p-state prewarm matmuls on framework const tiles
NWARM_CS = 6               # 256-row prewarm matmuls once consts landed

DT_MM = os.environ.get("BASS_MM_DTYPE", "bfloat16")


# ------------------------------------------------------------- filter design
def _design():
    eps = np.sqrt(10.0 ** (0.1 * RP) - 1.0)
    mu = np.arcsinh(1.0 / eps) / N_ORDER
    k = np.arange(1, N_ORDER + 1)
    theta = np.pi * (2 * k - 1) / (2 * N_ORDER)
    p = -np.sinh(mu) * np.sin(theta) + 1j * np.cosh(mu) * np.cos(theta)
    g = np.prod(-p).real
    if N_ORDER % 2 == 0:
        g /= np.sqrt(1.0 + eps**2)
    fs = 2.0
    warped = 2.0 * fs * np.tan(np.pi * (0.8 / Q) / fs)
    p = p * warped
    g = g * warped**N_ORDER
    fs2 = 2.0 * fs
    pd = (fs2 + p) / (fs2 - p)
    zd = -np.ones(N_ORDER)
    gd = g * np.real(1.0 / np.prod(fs2 - p))
    b = np.real(gd * np.poly(zd))
    a = np.real(np.poly(pd))
    n = len(a)
    comp = np.zeros((n - 1, n - 1))
    comp[0, :] = -a[1:] / a[0]
    comp[1:, :-1] = np.eye(n - 2)
    IminusA = np.eye(n - 1) - comp.T
    B = b[1:] - a[1:] * b[0]
    zi = np.linalg.solve(IminusA, B)
    return b, a, zi


def _lfilter(b, a, x, zi):
    """Direct-form II transposed; x: (T, M) float64."""
    z = zi.copy()
    y = np.empty_like(x)
    for t in range(x.shape[0]):
        xt = x[t]
        yt = b[0] * xt + z[0]
        y[t] = yt
        z = np.concatenate([z[1:], np.zeros_like(z[:1])], axis=0) \
            + b[1:, None] * xt[None, :] - a[1:, None] * yt[None, :]
    return y


def _build_weights():
    """Returns (W_main [128, NIDX, 128], W_edge [128, KK, 128]) float64."""
    b, a, zi = _design()
    x = np.zeros((P, 1))
    x[0, 0] = 1.0
    h = _lfilter(b, a, x, np.zeros((N_ORDER, 1)))[:, 0]
    g = np.correlate(h, h, mode="full")            # 2P-1 taps
    G = np.zeros(NPH * PTAP)
    G[: 2 * P - 1] = g

    # main Toeplitz matrices: lhsT[q, r*NDELTA+d, m] = G[4*(q + 128*d - m) + r]
    m = np.arange(128)
    q = np.arange(128)
    W_main = np.zeros((128, NIDX, 128))
    for r in range(NPH):
        Gr = G[r::NPH]                             # PTAP taps
        for d in range(NDELTA):
            mp = q[:, None] + 128 * d - m[None, :]  # [q, m]
            valid = (mp >= 0) & (mp < PTAP)
            W_main[:, r * NDELTA + d, :] = np.where(valid, Gr[np.clip(mp, 0, PTAP - 1)], 0.0)

    # edge matrix: exact last-128 outputs as linear map of last W_EDGE ext samples
    t_idx = np.arange(W_EDGE)
    w_idx = np.arange(W_EDGE)
    d_idx = t_idx[:, None] - w_idx[None, :]
    hmat = np.where((d_idx >= 0) & (d_idx < P), h[np.clip(d_idx, 0, P - 1)], 0.0)
    y1 = hmat                                      # [t, w] forward FIR basis
    y1_rev = y1[::-1]
    z0 = zi[:, None] * y1_rev[0][None, :]
    y2 = _lfilter(b, a, y1_rev, z0)[::-1]
    S0 = L0 - W_EDGE
    js = np.arange(NOUT - 128, NOUT)
    ts = EDGE + 4 * js - S0
    M_edge = y2[ts, :]                             # [128, W_EDGE]
    W_edge = M_edge.reshape(128, KK, 128).transpose(2, 1, 0)  # [q, kk, j]
    return W_main, W_edge


_CACHE = {}


def _np_dt():
    if DT_MM == "bfloat16":
        import ml_dtypes
        return ml_dtypes.bfloat16
    return np.float32


def _prep_static():
    if "w" not in _CACHE:
        W_main, W_edge = _build_weights()
        dt = _np_dt()
        _CACHE["w"] = (np.ascontiguousarray(W_main, dt),
                       np.ascontiguousarray(W_edge, dt))
    return _CACHE["w"]


# ------------------------------------------------------------- bass program
def _make_tile_context_cls():
    from concourse.tile import TileContext
    from concourse.vector_clock import ScopedClock, VectorClock

    class SplitDrainTileContext(TileContext):
        """This walrus build allows very few attached sync-waits per
        instruction; the stock kernel-tail drain carries one wait per DMA
        lane/engine and gets rejected.  Split it into one drain per proc,
        each with a single wait."""

        def _drain_and_barrier(self, tick_clock, wait_clock):
            gc = tick_clock.global_clock
            n = len(gc)
            for proc in range(n):
                if gc[proc] == 0:
                    continue
                vec = [0] * n
                vec[proc] = gc[proc]
                d = self.nc.sync.drain()
                wait_clock.add_sem_waits(d.ins, ScopedClock({None: VectorClock(vec)}))
            self.nc.sync.drain()
            self.nc.all_engine_barrier()
            assert self.sems is not None
            popped = self.nc._tile_sem_poison_stack.pop()
            assert popped is self._sem_poison
            self.nc.clear_and_free_semaphores(list(self.sems.allocated().values()))
            self.nc.all_engine_barrier()

    return SplitDrainTileContext


def _build_nc():
    import concourse.bass as bass
    import concourse.mybir as mybir
    TileContext = _make_tile_context_cls()

    dt_mm = getattr(mybir.dt, DT_MM)
    f32 = mybir.dt.float32

    # consts layout along free dim: [wmain 8*128 | wedge 6*128 | etail 6*64]
    CW = NIDX * 128 + KK * 128 + KK * S            # 2176
    OFF_WE = NIDX * 128
    OFF_ET = OFF_WE + KK * 128

    nc = bass.Bass(target_bir_lowering=False)
    v_d = nc.declare_dram_parameter("v", [128, NPH, NB, S], dt_mm, isOutput=False)
    c_d = nc.declare_dram_parameter("consts", [128, CW], dt_mm, isOutput=False)
    out_d = nc.declare_dram_parameter("out", [128, HP * COLS_PER_HP, S], dt_mm,
                                      isOutput=True)

    # Walrus (this version) allows at most ONE attached sync-wait per
    # instruction, so the structure below arranges that every instruction
    # has at most one un-observed dependency:
    #  - piece/osb pools are deep enough to never recycle a slot
    #  - psum groups alternate two 3-deep banks; before a recycled bank's
    #    first real matmul (which waits on the piece DMA), a zero-valued
    #    start matmul merged into the same accumulation group absorbs the
    #    slot-release (DVE) wait
    with TileContext(nc) as tc:
        with tc.tile_pool(name="const", bufs=1) as cpool, \
             tc.tile_pool(name="vchunk", bufs=len(PIECES)) as vpool, \
             tc.tile_pool(name="osb", bufs=HP) as opool, \
             tc.tile_pool(name="psum", bufs=1, space="PSUM") as ppool:

            cs = cpool.tile([128, CW], dt_mm, tag="cs")
            edge_sb = cpool.tile([128, S], dt_mm, tag="edge")
            # zeros for the psum-resetting gate matmuls: memset on the DVE
            # instead of shipping them over HBM; the gates' slot-release
            # (DVE) wait transitively covers this write
            zs = cpool.tile([128, BLK * S], dt_mm, tag="zeros")
            nc.vector.memset(zs[:], 0.0)

            def piece_dma(k):
                c0_, ncols_ = PIECES[k]
                pc = vpool.tile([128, NPH, ncols_, S], dt_mm, tag="chunk")
                nc.sync.dma_start(out=pc[:], in_=v_d[:, :, c0_:c0_ + ncols_, :])
                return pc

            # input stream order: the main-path weights, then half-pass 0's
            # two half-pieces, then the edge/zeros consts, then the rest —
            # so the first matmul starts as early as possible
            nc.sync.dma_start(out=cs[:, :OFF_WE], in_=c_d[:, :OFF_WE])
            pieces = [piece_dma(0), piece_dma(1)]
            nc.sync.dma_start(out=cs[:, OFF_WE:], in_=c_d[:, OFF_WE:])
            for k in range(2, len(PIECES)):
                pieces.append(piece_dma(k))

            def wm(idx):
                return cs[:, idx * 128:(idx + 1) * 128]

            def we(kk):
                return cs[:, OFF_WE + kk * 128: OFF_WE + (kk + 1) * 128]

            def et(kk):
                return cs[:, OFF_ET + kk * S: OFF_ET + (kk + 1) * S]

            # p-state prewarm: tiny matmuls on the framework const tiles run
            # during the DMA of the weights; 256-row matmuls keep the PE busy
            # until piece 0 lands
            import concourse.mybir as mybir_
            czero = nc.const_aps.aps[(mybir_.dt.float32, 0.0)]
            warm = ppool.tile([128, 256], f32, tag="warm", bufs=1)
            for _ in range(NWARM_TINY):
                nc.tensor.matmul(warm[0:1, 0:1], czero, czero,
                                 start=True, stop=True)
            for _ in range(NWARM_CS):
                nc.tensor.matmul(warm[:], wm(0), cs[:, 0:256],
                                 start=True, stop=True)
            eps = ppool.tile([128, S], f32, tag="eps", bufs=1)

            # hp j -> [(piece, local col offset) for groups A, B]
            plan = [[(pieces[0], 0), (pieces[1], 0)]]
            for j in range(1, HP - 1):
                plan.append([(pieces[j + 1], 0), (pieces[j + 1], BLK)])
            plan.append([(pieces[HP], 0), (pieces[HP + 1], 0)])

            ob = None
            for j in range(HP):
                (pcA, lbA), (pcB, lbB) = plan[j]
                psA = ppool.tile([128, BLK, S], f32, tag="psA", name=f"psA{j}",
                                 bufs=3)
                psB = ppool.tile([128, BLK, S], f32, tag="psB", name=f"psB{j}",
                                 bufs=3)
                # outputs stage in 32-col pair tiles (hp0..5) or 16-col tiles
                # (hp6/7) so the gpsimd software queue carries only 6 DMAs
                if j < 6:
                    if j % 2 == 0:
                        ob = opool.tile([128, 2 * COLS_PER_HP, S], dt_mm,
                                        tag="osb32", bufs=3)
                    off = (j % 2) * COLS_PER_HP
                else:
                    ob = opool.tile([128, COLS_PER_HP, S], dt_mm, tag="osb16",
                                    bufs=2)
                    off = 0
                recycled = j >= 3
                if recycled:
                    nc.tensor.matmul(psA[:], wm(0), zs[:], start=True,
                                     stop=False, skip_group_check=True)
                    nc.tensor.matmul(psB[:], wm(0), zs[:], start=True,
                                     stop=False, skip_group_check=True)
                # hp0/hp7's halves read different pieces: emit sequentially so
                # the A chain never stalls on the B piece
                halves = ((0, psA, pcA, lbA), (1, psB, pcB, lbB))
                if j in (0, HP - 1):
                    order = [(idx, h) for h in halves for idx in range(NIDX)]
                else:
                    order = [(idx, h) for idx in range(NIDX) for h in halves]
                for idx, (half, ps, pc, lb) in order:
                    r, dd = divmod(idx, NDELTA)
                    c0 = lb + dd
                    nc.tensor.matmul(
                        ps[:], wm(idx), pc[:, r, c0:c0 + BLK, :],
                        start=(idx == 0 and not recycled),
                        stop=(idx == NIDX - 1),
                        skip_group_check=True)
                if j == 1:
                    # exact right-edge outputs (out column n=127): its consts
                    # arrive mid-stream, after half-pass 0's pieces
                    for kk in range(KK):
                        nc.tensor.matmul(eps[:], we(kk), et(kk),
                                         start=(kk == 0), stop=(kk == KK - 1))
                    nc.vector.tensor_copy(edge_sb[:], eps[:])
                nc.vector.tensor_copy(ob[:, off:off + BLK, :], psA[:])
                if j == HP - 1:
                    # first half of the final output leaves as soon as its
                    # copy lands; the rest follows after the edge column
                    nc.gpsimd.dma_start(
                        out=out_d[:, 112:112 + BLK, :], in_=ob[:, 0:BLK, :])
                nc.vector.tensor_copy(ob[:, off + BLK:off + COLS_PER_HP, :],
                                      psB[:])
                if j == HP - 1:
                    nc.vector.tensor_copy(ob[:, COLS_PER_HP - 1, :], edge_sb[:])
                    nc.gpsimd.dma_start(
                        out=out_d[:, 120:128, :], in_=ob[:, BLK:COLS_PER_HP, :])
                elif j == 6:
                    nc.gpsimd.dma_start(out=out_d[:, 96:112, :], in_=ob[:])
                elif j % 2 == 1:
                    n0 = 2 * COLS_PER_HP * (j // 2)
                    nc.gpsimd.dma_start(
                        out=out_d[:, n0:n0 + 2 * COLS_PER_HP, :], in_=ob[:])
    return nc


# --------------------------------------------------------------- host paths
def _host_prep(x):
    """x: (32, 16, T) float32 -> per-core input maps."""
    W_main, W_edge = _prep_static()
    dt = _np_dt()
    xs = np.asarray(x, np.float32).reshape(NCORES * S, T)
    left = 2.0 * xs[:, :1] - xs[:, EDGE:0:-1]
    right = 2.0 * xs[:, -1:] - xs[:, -2:-(EDGE + 2):-1]
    ext = np.concatenate([left, xs, right], axis=1)          # (512, L0)
    u = np.empty((NCORES * S, ULEN), np.float32)
    u[:, :LPAD] = ext[:, :1]
    u[:, LPAD:LPAD + L0] = ext
    u[:, LPAD + L0:] = ext[:, -1:]
    in_maps = []
    for c in range(NCORES):
        uc = u[c * S:(c + 1) * S]                            # (64, ULEN)
        # V[q, r, n, s] = u[s, 4*(q + 128*n) + r]
        u6 = uc.reshape(S, NB, 128, NPH)                     # [s, n, q, r]
        V = np.ascontiguousarray(u6.transpose(2, 3, 1, 0), dt)  # [q, r, n, s]
        etc = ext[c * S:(c + 1) * S, -W_EDGE:]               # (64, W_EDGE)
        etail = np.ascontiguousarray(
            etc.T.reshape(KK, 128, S).transpose(1, 0, 2))    # [q, kk, s]
        consts = np.concatenate(
            [W_main.reshape(128, NIDX * 128).astype(np.float32),
             W_edge.reshape(128, KK * 128).astype(np.float32),
             etail.reshape(128, KK * S)], axis=1)            # [128, 2176]
        in_maps.append({"v": V, "consts": np.ascontiguousarray(consts, dt)})
    return in_maps


def _host_post(results):
    ys = []
    for c in range(NCORES):
        o = np.asarray(results[c]["out"], dtype=np.float32)  # [128 m, 128 n, 64 s]
        ys.append(np.ascontiguousarray(o.transpose(2, 1, 0)).reshape(S, NOUT))
    return np.concatenate(ys, axis=0).reshape(32, 16, NOUT).astype(np.float32)


def _get_nc():
    if "nc" not in _CACHE:
        _CACHE["nc"] = _build_nc()
    return _CACHE["nc"]


def kernel(x, _trace=False, _trace_kwargs=None):
    from concourse.bass_utils import run_bass_kernel_spmd
    nc = _get_nc()
    in_maps = _host_prep(x)
    res = run_bass_kernel_spmd(nc, in_maps, list(range(NCORES)),
                               trace=_trace, **(_trace_kwargs or {}))
    out = _host_post(res.results)
    if _trace:
        _CACHE["last_exec_time_ns"] = res.exec_time_ns
        _CACHE["last_result"] = res
    return out


# revision 43
# speedup vs baseline: 1.0215x; 1.0189x over previous
"""Trainium2 kernel for BandDecimate: scipy.signal.decimate(x, q=4, n=8,
ftype='iir', zero_phase=True) on x of shape (32, 16, 65536).

Method: filtfilt with the order-8 Chebyshev-I filter is re-expressed as a
single symmetric FIR g = corr(h, h) (h = 256-tap truncated impulse
response; slowest pole |p|=0.958 -> tail ~2e-5, far below the 2e-2
tolerance), applied to the odd-extended, steady-state-padded signal,
fused with the decimation by 4 via a 4-phase polyphase decomposition.
Each phase is a 128-tap correlation computed on the PE array as
block-Toeplitz matmuls in bf16 (halves both HBM traffic and weight-load
time vs fp32; rel-err ~3.5e-3 incl. quantization).  The right edge (last
128 decimated outputs), where the backward-pass initial condition
differs from the symmetric-FIR approximation, is computed exactly by a
dense precomputed 128x768 linear map (6 extra matmuls).  512 independent
series are sharded 64-per-core across 8 cores.

Schedule: weights first, then the input V matrix streams in 10 pieces
(the first and last half-passes' pieces split in two) so the first
matmul starts right after ~0.85MB has landed and the final output ships
while the last psum group still computes.  Dummy matmuls (tiny ones on
the framework const tiles, then 256-row ones on the weights) keep the
PE busy through its p-state ramp until piece 0 lands.  Outputs stage
through bf16 SBUF tiles and leave on the gpsimd software queues so they
never steal input bandwidth.
"""
import os
import sys

import numpy as np

sys.path.insert(0, "/opt/trn_rl_repo")

# ---------------------------------------------------------------- constants
Q = 4
N_ORDER = 8
RP = 0.05
T = 65536
EDGE = 27
L0 = T + 2 * EDGE          # 65590
P = 256                    # truncated IIR impulse response length
NPH = 4                    # polyphase phases
PTAP = 2 * P // NPH        # 128 taps per phase
LPAD = P - 1 - EDGE        # 228 left steady-state pad of u
NB = 129                   # V column blocks (ULEN = 512*NB)
ULEN = 512 * NB            # 66048
RPAD = ULEN - LPAD - L0    # 230 right pad (clamp; fixed by edge path)
NOUT = T // Q              # 16384 outputs per series
S = 64                     # series per core
NCORES = 8
W_EDGE = 768               # edge window length
KK = W_EDGE // 128         # 6
NDELTA = 2
NIDX = NPH * NDELTA        # 8 toeplitz matrices
BLK = 8                    # output columns per psum group (free = BLK*S = 512)
HP = 8                     # half-passes (two psum groups each)
COLS_PER_HP = 2 * BLK      # 16 output n-columns per half-pass
PIECE_N = COLS_PER_HP + NDELTA - 1  # 17 V columns per piece
# half-pass 0's piece is split 9+8 columns so its first psum group starts
# one half-piece earlier; half-pass 7's likewise so its first output leaves
# while the second half still computes
PIECES = ([(0, 9), (8, 9)] + [(16 * j, 17) for j in range(1, HP - 1)]
          + [(112, 9), (120, 9)])
NWARM_TINY = 24            # p-state prewarm matmuls on framework const tiles
NWARM_CS = 6               # 256-row prewarm matmuls once consts landed

DT_MM = os.environ.get("BASS_MM_DTYPE", "bfloat16")


# ------------------------------------------------------------- filter design
def _design():
    eps = np.sqrt(10.0 ** (0.1 * RP) - 1.0)
    mu = np.arcsinh(1.0 / eps) / N_ORDER
    k = np.arange(1, N_ORDER + 1)
    theta = np.pi * (2 * k - 1) / (2 * N_ORDER)
    p = -np.sinh(mu) * np.sin(theta) + 1j * np.cosh(mu) * np.cos(theta)
    g = np.prod(-p).real
    if N_ORDER % 2 == 0:
        g /= np.sqrt(1.0 + eps**2)
    fs = 2.0
    warped = 2.0 * fs * np.tan(np.pi * (0.8 / Q) / fs)
    p = p * warped
    g = g * warped**N_ORDER
    fs2 = 2.0 * fs
    pd = (fs2 + p) / (fs2 - p)
    zd = -np.ones(N_ORDER)
    gd = g * np.real(1.0 / np.prod(fs2 - p))
    b = np.real(gd * np.poly(zd))
    a = np.real(np.poly(pd))
    n = len(a)
    comp = np.zeros((n - 1, n - 1))
    comp[0, :] = -a[1:] / a[0]
    comp[1:, :-1] = np.eye(n - 2)
    IminusA = np.eye(n - 1) - comp.T
    B = b[1:] - a[1:] * b[0]
    zi = np.linalg.solve(IminusA, B)
    return b, a, zi


def _lfilter(b, a, x, zi):
    """Direct-form II transposed; x: (T, M) float64."""
    z = zi.copy()
    y = np.empty_like(x)
    for t in range(x.shape[0]):
        xt = x[t]
        yt = b[0] * xt + z[0]
        y[t] = yt
        z = np.concatenate([z[1:], np.zeros_like(z[:1])], axis=0) \
            + b[1:, None] * xt[None, :] - a[1:, None] * yt[None, :]
    return y


def _build_weights():
    """Returns (W_main [128, NIDX, 128], W_edge [128, KK, 128]) float64."""
    b, a, zi = _design()
    x = np.zeros((P, 1))
    x[0, 0] = 1.0
    h = _lfilter(b, a, x, np.zeros((N_ORDER, 1)))[:, 0]
    g = np.correlate(h, h, mode="full")            # 2P-1 taps
    G = np.zeros(NPH * PTAP)
    G[: 2 * P - 1] = g

    # main Toeplitz matrices: lhsT[q, r*NDELTA+d, m] = G[4*(q + 128*d - m) + r]
    m = np.arange(128)
    q = np.arange(128)
    W_main = np.zeros((128, NIDX, 128))
    for r in range(NPH):
        Gr = G[r::NPH]                             # PTAP taps
        for d in range(NDELTA):
            mp = q[:, None] + 128 * d - m[None, :]  # [q, m]
            valid = (mp >= 0) & (mp < PTAP)
            W_main[:, r * NDELTA + d, :] = np.where(valid, Gr[np.clip(mp, 0, PTAP - 1)], 0.0)

    # edge matrix: exact last-128 outputs as linear map of last W_EDGE ext samples
    t_idx = np.arange(W_EDGE)
    w_idx = np.arange(W_EDGE)
    d_idx = t_idx[:, None] - w_idx[None, :]
    hmat = np.where((d_idx >= 0) & (d_idx < P), h[np.clip(d_idx, 0, P - 1)], 0.0)
    y1 = hmat                                      # [t, w] forward FIR basis
    y1_rev = y1[::-1]
    z0 = zi[:, None] * y1_rev[0][None, :]
    y2 = _lfilter(b, a, y1_rev, z0)[::-1]
    S0 = L0 - W_EDGE
    js = np.arange(NOUT - 128, NOUT)
    ts = EDGE + 4 * js - S0
    M_edge = y2[ts, :]                             # [128, W_EDGE]
    W_edge = M_edge.reshape(128, KK, 128).transpose(2, 1, 0)  # [q, kk, j]
    return W_main, W_edge


_CACHE = {}


def _np_dt():
    if DT_MM == "bfloat16":
        import ml_dtypes
        return ml_dtypes.bfloat16
    return np.float32


def _prep_static():
    if "w" not in _CACHE:
        W_main, W_edge = _build_weights()
        dt = _np_dt()
        _CACHE["w"] = (np.ascontiguousarray(W_main, dt),
                       np.ascontiguousarray(W_edge, dt))
    return _CACHE["w"]


# ------------------------------------------------------------- bass program
def _make_tile_context_cls():
    from concourse.tile import TileContext
    from concourse.vector_clock import ScopedClock, VectorClock

    class SplitDrainTileContext(TileContext):
        """This walrus build allows very few attached sync-waits per
        instruction; the stock kernel-tail drain carries one wait per DMA
        lane/engine and gets rejected.  Split it into one drain per proc,
        each with a single wait."""

        def _drain_and_barrier(self, tick_clock, wait_clock):
            gc = tick_clock.global_clock
            n = len(gc)
            for proc in range(n):
                if gc[proc] == 0:
                    continue
                vec = [0] * n
                vec[proc] = gc[proc]
                d = self.nc.sync.drain()
                wait_clock.add_sem_waits(d.ins, ScopedClock({None: VectorClock(vec)}))
            self.nc.sync.drain()
            self.nc.all_engine_barrier()
            assert self.sems is not None
            popped = self.nc._tile_sem_poison_stack.pop()
            assert popped is self._sem_poison
            self.nc.clear_and_free_semaphores(list(self.sems.allocated().values()))
            self.nc.all_engine_barrier()

    return SplitDrainTileContext


def _build_nc():
    import concourse.bass as bass
    import concourse.mybir as mybir
    TileContext = _make_tile_context_cls()

    dt_mm = getattr(mybir.dt, DT_MM)
    f32 = mybir.dt.float32

    # consts layout along free dim: [wmain 8*128 | wedge 6*128 | etail 6*64]
    CW = NIDX * 128 + KK * 128 + KK * S            # 2176
    OFF_WE = NIDX * 128
    OFF_ET = OFF_WE + KK * 128

    nc = bass.Bass(target_bir_lowering=False)
    v_d = nc.declare_dram_parameter("v", [128, NPH, NB, S], dt_mm, isOutput=False)
    c_d = nc.declare_dram_parameter("consts", [128, CW], dt_mm, isOutput=False)
    out_d = nc.declare_dram_parameter("out", [128, HP * COLS_PER_HP, S], dt_mm,
                                      isOutput=True)

    # Walrus (this version) allows at most ONE attached sync-wait per
    # instruction, so the structure below arranges that every instruction
    # has at most one un-observed dependency:
    #  - piece/osb pools are deep enough to never recycle a slot
    #  - psum groups alternate two 3-deep banks; before a recycled bank's
    #    first real matmul (which waits on the piece DMA), a zero-valued
    #    start matmul merged into the same accumulation group absorbs the
    #    slot-release (DVE) wait
    with TileContext(nc) as tc:
        with tc.tile_pool(name="const", bufs=1) as cpool, \
             tc.tile_pool(name="vchunk", bufs=len(PIECES)) as vpool, \
             tc.tile_pool(name="osb", bufs=HP) as opool, \
             tc.tile_pool(name="psum", bufs=1, space="PSUM") as ppool:

            cs = cpool.tile([128, CW], dt_mm, tag="cs")
            edge_sb = cpool.tile([128, S], dt_mm, tag="edge")
            # zeros for the psum-resetting gate matmuls: memset on the DVE
            # instead of shipping them over HBM; the gates' slot-release
            # (DVE) wait transitively covers this write
            zs = cpool.tile([128, BLK * S], dt_mm, tag="zeros")
            nc.vector.memset(zs[:], 0.0)

            def piece_dma(k):
                c0_, ncols_ = PIECES[k]
                pc = vpool.tile([128, NPH, ncols_, S], dt_mm, tag="chunk")
                nc.sync.dma_start(out=pc[:], in_=v_d[:, :, c0_:c0_ + ncols_, :])
                return pc

            # input stream order: the main-path weights, then half-pass 0's
            # two half-pieces, then the edge/zeros consts, then the rest —
            # so the first matmul starts as early as possible
            nc.sync.dma_start(out=cs[:, :OFF_WE], in_=c_d[:, :OFF_WE])
            pieces = [piece_dma(0), piece_dma(1)]
            nc.sync.dma_start(out=cs[:, OFF_WE:], in_=c_d[:, OFF_WE:])
            for k in range(2, len(PIECES)):
                pieces.append(piece_dma(k))

            def wm(idx):
                return cs[:, idx * 128:(idx + 1) * 128]

            def we(kk):
                return cs[:, OFF_WE + kk * 128: OFF_WE + (kk + 1) * 128]

            def et(kk):
                return cs[:, OFF_ET + kk * S: OFF_ET + (kk + 1) * S]

            # p-state prewarm: tiny matmuls on the framework const tiles run
            # during the DMA of the weights; 256-row matmuls keep the PE busy
            # until piece 0 lands
            import concourse.mybir as mybir_
            czero = nc.const_aps.aps[(mybir_.dt.float32, 0.0)]
            warm = ppool.tile([128, 256], f32, tag="warm", bufs=1)
            for _ in range(NWARM_TINY):
                nc.tensor.matmul(warm[0:1, 0:1], czero, czero,
                                 start=True, stop=True)
            for _ in range(NWARM_CS):
                nc.tensor.matmul(warm[:], wm(0), cs[:, 0:256],
                                 start=True, stop=True)
            eps = ppool.tile([128, S], f32, tag="eps", bufs=1)

            # hp j -> [(piece, local col offset) for groups A, B]
            plan = [[(pieces[0], 0), (pieces[1], 0)]]
            for j in range(1, HP - 1):
                plan.append([(pieces[j + 1], 0), (pieces[j + 1], BLK)])
            plan.append([(pieces[HP], 0), (pieces[HP + 1], 0)])

            ob = None
            for j in range(HP):
                (pcA, lbA), (pcB, lbB) = plan[j]
                psA = ppool.tile([128, BLK, S], f32, tag="psA", name=f"psA{j}",
                                 bufs=3)
                psB = ppool.tile([128, BLK, S], f32, tag="psB", name=f"psB{j}",
                                 bufs=3)
                # outputs stage in 32-col pair tiles (hp0..5) or 16-col tiles
                # (hp6/7) so the gpsimd software queue carries only 6 DMAs
                if j < 6:
                    if j % 2 == 0:
                        ob = opool.tile([128, 2 * COLS_PER_HP, S], dt_mm,
                                        tag="osb32", bufs=3)
                    off = (j % 2) * COLS_PER_HP
                else:
                    ob = opool.tile([128, COLS_PER_HP, S], dt_mm, tag="osb16",
                                    bufs=2)
                    off = 0
                recycled = j >= 3
                if recycled:
                    nc.tensor.matmul(psA[:], wm(0), zs[:], start=True,
                                     stop=False, skip_group_check=True)
                    nc.tensor.matmul(psB[:], wm(0), zs[:], start=True,
                                     stop=False, skip_group_check=True)
                # hp0/hp7's halves read different pieces: emit sequentially so
                # the A chain never stalls on the B piece
                halves = ((0, psA, pcA, lbA), (1, psB, pcB, lbB))
                if j in (0, HP - 1):
                    order = [(idx, h) for h in halves for idx in range(NIDX)]
                else:
                    order = [(idx, h) for idx in range(NIDX) for h in halves]
                for idx, (half, ps, pc, lb) in order:
                    r, dd = divmod(idx, NDELTA)
                    c0 = lb + dd
                    nc.tensor.matmul(
                        ps[:], wm(idx), pc[:, r, c0:c0 + BLK, :],
                        start=(idx == 0 and not recycled),
                        stop=(idx == NIDX - 1),
                        skip_group_check=True)
                if j == 1:
                    # exact right-edge outputs (out column n=127): its consts
                    # arrive mid-stream, after half-pass 0's pieces
                    for kk in range(KK):
                        nc.tensor.matmul(eps[:], we(kk), et(kk),
                                         start=(kk == 0), stop=(kk == KK - 1))
                    nc.vector.tensor_copy(edge_sb[:], eps[:])
                nc.vector.tensor_copy(ob[:, off:off + BLK, :], psA[:])
                if j == HP - 1:
                    # first half of the final output leaves as soon as its
                    # copy lands; the rest follows after the edge column
                    nc.gpsimd.dma_start(
                        out=out_d[:, 112:112 + BLK, :], in_=ob[:, 0:BLK, :])
                nc.vector.tensor_copy(ob[:, off + BLK:off + COLS_PER_HP, :],
                                      psB[:])
                if j == HP - 1:
                    nc.vector.tensor_copy(ob[:, COLS_PER_HP - 1, :], edge_sb[:])
                    nc.gpsimd.dma_start(
                        out=out_d[:, 120:128, :], in_=ob[:, BLK:COLS_PER_HP, :])
                elif j == 6:
                    nc.gpsimd.dma_start(out=out_d[:, 96:112, :], in_=ob[:])
                elif j % 2 == 1:
                    n0 = 2 * COLS_PER_HP * (j // 2)
                    nc.gpsimd.dma_start(
                        out=out_d[:, n0:n0 + 2 * COLS_PER_HP, :], in_=ob[:])
    return nc


# --------------------------------------------------------------- host paths
def _host_prep(x):
    """x: (32, 16, T) float32 -> per-core input maps."""
    W_main, W_edge = _prep_static()
    dt = _np_dt()
    xs = np.asarray(x, np.float32).reshape(NCORES * S, T)
    left = 2.0 * xs[:, :1] - xs[:, EDGE:0:-1]
    right = 2.0 * xs[:, -1:] - xs[:, -2:-(EDGE + 2):-1]
    ext = np.concatenate([left, xs, right], axis=1)          # (512, L0)
    u = np.empty((NCORES * S, ULEN), np.float32)
    u[:, :LPAD] = ext[:, :1]
    u[:, LPAD:LPAD + L0] = ext
    u[:, LPAD + L0:] = ext[:, -1:]
    in_maps = []
    for c in range(NCORES):
        uc = u[c * S:(c + 1) * S]                            # (64, ULEN)
        # V[q, r, n, s] = u[s, 4*(q + 128*n) + r]
        u6 = uc.reshape(S, NB, 128, NPH)                     # [s, n, q, r]
        V = np.ascontiguousarray(u6.transpose(2, 3, 1, 0), dt)  # [q, r, n, s]
        etc = ext[c * S:(c + 1) * S, -W_EDGE:]               # (64, W_EDGE)
        etail = np.ascontiguousarray(
            etc.T.reshape(KK, 128, S).transpose(1, 0, 2))    # [q, kk, s]
        consts = np.concatenate(
            [W_main.reshape(128, NIDX * 128).astype(np.float32),
             W_edge.reshape(128, KK * 128).astype(np.float32),
             etail.reshape(128, KK * S)], axis=1)            # [128, 2176]
        in_maps.append({"v": V, "consts": np.ascontiguousarray(consts, dt)})
    return in_maps


def _host_post(results):
    ys = []
    for c in range(NCORES):
        o = np.asarray(results[c]["out"], dtype=np.float32)  # [128 m, 128 n, 64 s]
        ys.append(np.ascontiguousarray(o.transpose(2, 1, 0)).reshape(S, NOUT))
    return np.concatenate(ys, axis=0).reshape(32, 16, NOUT).astype(np.float32)


def _get_nc():
    if "nc" not in _CACHE:
        _CACHE["nc"] = _build_nc()
    return _CACHE["nc"]


def kernel(x, _trace=False, _trace_kwargs=None):
    from concourse.bass_utils import run_bass_kernel_spmd
    nc = _get_nc()
    in_maps = _host_prep(x)
    res = run_bass_kernel_spmd(nc, in_maps, list(range(NCORES)),
                               trace=_trace, **(_trace_kwargs or {}))
    out = _host_post(res.results)
    if _trace:
        _CACHE["last_exec_time_ns"] = res.exec_time_ns
        _CACHE["last_result"] = res
    return out
